# revision 20
# baseline (speedup 1.0000x reference)
"""CRF Viterbi decode kernel for Trainium2 (Bass), data-parallel over batch.

Problem shapes (hardcoded): X [32,128,10000] f32 one-hot, t_feats [48,48],
e_feats [48,10000].  Output Y_hat [32,128,48] f32 one-hot.

Sharding: batch 32 -> 8 cores x 4.  t_feats / e_feats replicated.

Per-core pipeline (4 batch elems, two scan "chains" of 2 elems on 96
partitions):
  1. emissions em^T[l,(b,t)] = e_feats @ X_b^T: per 128-wide V chunk, DMA X
     naturally [t,(b,v)], PE-transpose to [v,t], matmul accumulate in PSUM.
  2. Viterbi scan over t: scores = blockdiag(t_feats) + bcast(delta col) via
     two accumulating PE matmuls; DVE max (top-8) + max_index (backpointers);
     ScalarE adds the emission column -> next delta column.
  3. n_maxs/n_idxs/end_n/start_t via max/max_index + PE transposes.
  4. backtrace: copy_predicated carry, one-hot write into Y slab, dot with
     backpointer row (mult + reduce) to chase the pointer.
"""

import os
import sys

import numpy as np

for _p in ("/opt/trn_rl_repo",):
    if _p not in sys.path and os.path.isdir(_p):
        sys.path.insert(0, _p)

import concourse.bass as bass
import concourse.tile as tile
from concourse import mybir
from concourse.bass_utils import run_bass_kernel_spmd

F32 = mybir.dt.float32
U32 = mybir.dt.uint32
I32 = mybir.dt.int32
AL = mybir.AluOpType
AF = mybir.ActivationFunctionType

B, T, V, L = 32, 128, 10000, 48
NCORES = 8
BLOC = B // NCORES          # 4 batch elems per core
NCH = 2                     # scan chains per core (2 batch elems each)
P2 = 2 * L                  # 96 partitions per chain
NEG = -1.0e30               # block-diagonal mask value
NV0 = 78 * 128              # bulk V rows (chunks 0..77)
VCH = [128] * 78 + [16]     # V chunking (sum = 10000)

# packed constants layout: [128, CW] f32
# cols 0:128    ident (128 partitions)
# cols 128:224  blockdiag(t_feats) with NEG off-blocks (96 partitions)
# col  224      d0 column (96 partitions)
# col  225      offs column (96 partitions)
# cols 226:274  iota48 (4 partitions)
# cols 274:403  iota129 (4 partitions)
CW = 403


def build_nc():
    nc = bass.Bass()

    x = nc.declare_dram_parameter("x", [BLOC, T, V], F32, isOutput=False)
    eT = nc.declare_dram_parameter("eT", [V, L], F32, isOutput=False)
    consts = nc.declare_dram_parameter("consts", [128, CW], F32, isOutput=False)
    y = nc.declare_dram_parameter("y", [BLOC, T, L], F32, isOutput=True)

    with tile.TileContext(nc) as tc:
        from contextlib import ExitStack

        with ExitStack() as ctx:
            cons = ctx.enter_context(tc.tile_pool(name="cons", bufs=1))
            pers = ctx.enter_context(tc.tile_pool(name="pers", bufs=1))
            xpool = ctx.enter_context(tc.tile_pool(name="xpool", bufs=3))
            xtpool = ctx.enter_context(tc.tile_pool(name="xtpool", bufs=3))
            srpool = ctx.enter_context(tc.tile_pool(name="srpool", bufs=2))
            ppxt = ctx.enter_context(tc.tile_pool(name="ppxt", bufs=2, space="PSUM"))
            ppem = ctx.enter_context(tc.tile_pool(name="ppem", bufs=1, space="PSUM"))
            ppdel = ctx.enter_context(tc.tile_pool(name="ppdel", bufs=1, space="PSUM"))

            # ---- constants: ONE DMA ----
            cons_sb = cons.tile([128, CW], F32)
            nc.sync.dma_start(out=cons_sb, in_=consts[:, :])
            id_sb = cons_sb[:, 0:128]
            id96 = cons_sb[0:P2, 0:P2]
            tbd_sb = cons_sb[0:P2, 128:224]
            d0_sb = cons_sb[0:P2, 224:225]
            offs_sb = cons_sb[0:P2, 225:226]
            io48_sb = cons_sb[0:BLOC, 226:274]
            io129_sb = cons_sb[0:BLOC, 274:403]

            # ---- e_feats^T staged whole: 2 DMAs ----
            ef_bulk = cons.tile([128, 78, L], F32)
            nc.sync.dma_start(
                out=ef_bulk,
                in_=eT[0:NV0, :].rearrange("(k v) l -> v k l", v=128),
            )
            ef_tail = cons.tile([16, L], F32)
            nc.sync.dma_start(out=ef_tail, in_=eT[NV0:V, :])

            m8 = []
            bp8 = []
            em_sb = []
            dcols = []
            for c in range(NCH):
                m8.append(pers.tile([P2, 8], F32, name=f"m8_{c}"))
                bp8.append(pers.tile([P2, 8 * T], U32, name=f"bp8_{c}"))
                em_sb.append(pers.tile([P2, T], F32, name=f"em_{c}"))
                dcols.append(pers.tile([P2, T], F32, name=f"dcols_{c}"))

            # ---- phase 1: em^T[l, (b,t)] = e_feats @ X_b^T, PSUM accum ----
            psum_em = [
                ppem.tile([L, T], F32, name=f"psum_em_{b}", tag=f"pem_{b}")
                for b in range(BLOC)
            ]
            nchunks = len(VCH)
            vo = 0
            for k, vs in enumerate(VCH):
                x4 = xpool.tile([T, BLOC, 128], F32, name="x4")
                nc.sync.dma_start(
                    out=x4[:, :, :vs],
                    in_=x[:, :, vo : vo + vs].rearrange("b t v -> t b v"),
                )
                ef = ef_bulk[:, k, :] if k < 78 else ef_tail
                pxt = ppxt.tile([128, BLOC, T], F32, name="pxt")
                for b in range(BLOC):
                    nc.tensor.transpose(pxt[:vs, b, :], x4[:, b, :vs], id_sb)
                xt = xtpool.tile([128, BLOC, T], F32, name="xt")
                if k % 2 == 0:
                    nc.scalar.copy(out=xt[:vs], in_=pxt[:vs])
                else:
                    nc.vector.tensor_copy(xt[:vs], pxt[:vs])
                for b in range(BLOC):
                    nc.tensor.matmul(
                        psum_em[b],
                        ef[:vs, :],
                        xt[:vs, b, :],
                        start=(k == 0),
                        stop=(k == nchunks - 1),
                    )
                vo += vs

            # em rows -> [(b,l), t] per chain: PSUM->SBUF copy, then DMA remap
            em_tmp = pers.tile([L, BLOC, T], F32)
            for b in range(BLOC):
                nc.scalar.copy(out=em_tmp[:, b, :], in_=psum_em[b])
            em_raw = [
                pers.tile([P2, T], F32, name=f"em_raw_{c}") for c in range(NCH)
            ]
            for c in range(NCH):
                for bb in range(2):
                    nc.sync.dma_start(
                        out=em_raw[c][bb * L : (bb + 1) * L, :],
                        in_=em_tmp[:, 2 * c + bb, :],
                    )
                # funnel the two DMA waits onto one DVE copy
                nc.vector.tensor_copy(em_sb[c], em_raw[c])

            # ---- phase 2: Viterbi scan ----
            # scores[(b,j),(b,i)] = t_feats[i,j] + delta[b,i]: two PE matmuls
            for t in range(T):
                for c in range(NCH):
                    dcol = d0_sb if t == 0 else dcols[c][:, t - 1 : t]
                    psc = ppdel.tile(
                        [P2, P2], F32, name=f"psc_{c}", tag=f"psc_{c}"
                    )
                    nc.tensor.matmul(psc, tbd_sb, id96, start=True, stop=False)
                    nc.tensor.matmul(
                        psc,
                        dcol.broadcast_to([P2, P2]),
                        id96,
                        start=False,
                        stop=True,
                    )
                    nc.vector.max(m8[c], psc)
                    nc.vector.max_index(
                        out=bp8[c][:, 8 * t : 8 * t + 8],
                        in_max=m8[c],
                        in_values=psc,
                    )
                    nc.scalar.activation(
                        out=dcols[c][:, t : t + 1],
                        in_=m8[c][:, 0:1],
                        func=AF.Identity,
                        bias=em_sb[c][:, t : t + 1],
                        scale=1.0,
                    )

            # ---- phase 3: n_maxs / n_idxs / end_n / start_t ----
            delta_sb = pers.tile([T, BLOC * L], F32)
            for c in range(NCH):
                pdt = ppxt.tile([T, P2], F32, name="pdt", tag="pxt")
                nc.tensor.transpose(pdt, dcols[c], id96)
                nc.scalar.copy(out=delta_sb[:, c * P2 : (c + 1) * P2], in_=pdt)

            nmcols = pers.tile([T, BLOC], F32)
            nicols = pers.tile([T, BLOC], F32)
            for b in range(BLOC):
                mx8 = srpool.tile([T, 8], F32, name="mx8", tag="mx8")
                nc.vector.max(mx8, delta_sb[:, b * L : (b + 1) * L])
                ix8 = srpool.tile([T, 8], U32, name="ix8", tag="ix8")
                nc.vector.max_index(ix8, mx8, delta_sb[:, b * L : (b + 1) * L])
                nc.scalar.copy(out=nmcols[:, b : b + 1], in_=mx8[:, 0:1])
                nc.vector.tensor_copy(nicols[:, b : b + 1], ix8[:, 0:1])

            pnmi = ppxt.tile([BLOC, 2, T], F32, name="pnmi", tag="pxt")
            nc.tensor.transpose(pnmi[:, 0, :], nmcols, id_sb)
            nc.tensor.transpose(pnmi[:, 1, :], nicols, id_sb)
            nmb = pers.tile([BLOC, T + 1], F32)
            nib = pers.tile([BLOC, T + 1], F32)
            nc.vector.memset(nmb[:, 0:1], 0.0)
            nc.vector.memset(nib[:, 0:1], 0.0)
            nc.scalar.copy(out=nmb[:, 1:], in_=pnmi[:, 0, :])
            nc.scalar.copy(out=nib[:, 1:], in_=pnmi[:, 1, :])

            en8 = pers.tile([BLOC, 8], F32)
            nc.vector.max(en8, nmb)
            eni8 = pers.tile([BLOC, 8], U32)
            nc.vector.max_index(eni8, en8, nmb)
            endf = pers.tile([BLOC, 1], F32)
            nc.vector.tensor_copy(endf, eni8[:, 0:1])

            oh = pers.tile([BLOC, T + 1], F32)
            nc.vector.tensor_scalar(
                out=oh, in0=io129_sb, scalar1=endf, scalar2=None, op0=AL.is_equal
            )
            ohi = pers.tile([BLOC, T + 1], I32)
            nc.vector.tensor_copy(ohi, oh)
            act = pers.tile([BLOC, T + 1], F32)
            nc.vector.tensor_scalar(
                out=act, in0=io129_sb, scalar1=endf, scalar2=None, op0=AL.is_le
            )
            scr129 = pers.tile([BLOC, T + 1], F32)
            start_t = pers.tile([BLOC, 1], F32)
            nc.vector.tensor_mul(scr129, oh, nib)
            nc.vector.tensor_reduce(
                out=start_t, in_=scr129, axis=mybir.AxisListType.X, op=AL.add
            )

            # ---- backpointer repack: bp8 [(b,l), 8t] u32 -> bpn [b, (n,l)] f32
            bpn_raw = pers.tile([BLOC, (T + 1) * L], F32)
            bt_sb = [
                pers.tile([T, P2], F32, name=f"bt_sb_{c}") for c in range(NCH)
            ]
            for c in range(NCH):
                bpfc = srpool.tile([P2, T], F32, name="bpfc", tag="bpfc")
                nc.vector.tensor_scalar(
                    out=bpfc,
                    in0=bp8[c].rearrange("p (t e) -> p t e", e=8)[:, :, 0],
                    scalar1=offs_sb,
                    scalar2=None,
                    op0=AL.subtract,
                )
                pbt = ppxt.tile([T, P2], F32, name="pbt", tag="pxt")
                nc.tensor.transpose(pbt, bpfc, id96)
                nc.scalar.copy(out=bt_sb[c], in_=pbt)
            for b in range(BLOC):
                c, bb = divmod(b, 2)
                nc.sync.dma_start(
                    out=bpn_raw[b : b + 1, L:].rearrange("p (t l) -> p t l", l=L),
                    in_=bt_sb[c][:, bb * L : (bb + 1) * L],
                )
            # funnel: concentrate the 4 DMA waits on one copy, so the
            # backtrace loop waits on DVE only
            bpn = pers.tile([BLOC, (T + 1) * L], F32)
            nc.vector.tensor_copy(bpn[:, L:], bpn_raw[:, L:])

            # ---- phase 4: backtrace ----
            yslab = pers.tile([BLOC, (T + 1) * L], F32)
            tcar = pers.tile([BLOC, 1], F32)
            nc.vector.memset(tcar, 0.0)
            for n in range(T, 0, -1):
                nc.vector.copy_predicated(
                    out=tcar, mask=ohi[:, n : n + 1], data=start_t
                )
                nc.vector.tensor_scalar(
                    out=yslab[:, n * L : (n + 1) * L],
                    in0=io48_sb,
                    scalar1=tcar,
                    scalar2=act[:, n : n + 1],
                    op0=AL.is_equal,
                    op1=AL.mult,
                )
                if n > 1:
                    scr = srpool.tile([BLOC, L], F32, name="scr", tag="scr")
                    nc.vector.tensor_mul(
                        scr,
                        yslab[:, n * L : (n + 1) * L],
                        bpn[:, n * L : (n + 1) * L],
                    )
                    nc.vector.tensor_reduce(
                        out=tcar, in_=scr, axis=mybir.AxisListType.X, op=AL.add
                    )

            nc.sync.dma_start(
                out=y[:, :, :],
                in_=yslab[:, L:].rearrange("b (t l) -> b t l", l=L),
            )

    nc.finalize()
    _legalize_sync_waits(nc)
    return nc


def _legalize_sync_waits(nc):
    """This container's walrus accepts at most ONE sync wait per instruction.

    Split excess waits onto Drain instructions inserted just before the
    offending instruction (same engine, so the waits still complete before it
    issues; an idle-pipe Drain costs ~12ns).  Applied to the serialized BIR
    only -- CoreSim consumes the in-memory module and is unaffected.
    """
    import json as _json

    m = _json.loads(nc.to_json_bytes())
    for f in m["functions"]:
        for blk in f["blocks"]:
            out = []
            for ins in blk["instructions"]:
                si = ins.get("sync_info") or {}
                w = si.get("on_wait") or []
                if len(w) > 1:
                    for j, wx in enumerate(w[:-1]):
                        out.append(
                            {
                                "debug": ins.get("debug", 0),
                                "engine": ins["engine"],
                                "ins": [],
                                "outs": [],
                                "name": f"{ins['name']}-w{j}",
                                "opcode": "Drain",
                                "sync_info": {"on_update": [], "on_wait": [wx]},
                            }
                        )
                    si["on_wait"] = [w[-1]]
                out.append(ins)
            blk["instructions"] = out
    blob = _json.dumps(m).encode()
    nc.to_json_bytes = lambda: blob


def make_consts():
    f32 = np.float32
    c = np.zeros((128, CW), f32)
    c[:128, 0:128] = np.eye(128, dtype=f32)
    c[0:P2, 128:224] = NEG
    c[0:P2, 224] = NEG
    c[0, 224] = 0.0
    c[L, 224] = 0.0
    c[0:L, 225] = 0.0
    c[L:P2, 225] = float(L)
    c[0:BLOC, 226:274] = np.arange(L, dtype=f32)[None, :]
    c[0:BLOC, 274:403] = np.arange(T + 1, dtype=f32)[None, :]
    return c


def make_in_maps(X, t_feats, e_feats):
    f32 = np.float32
    X = np.ascontiguousarray(X, dtype=f32)
    t_feats = np.asarray(t_feats, dtype=f32)
    e_feats = np.asarray(e_feats, dtype=f32)
    c = make_consts()
    # scores matmul computes psc = tbd_sb.T @ id, so store blockdiag(t_feats)
    c[0:L, 128 : 128 + L] = t_feats
    c[L:P2, 128 + L : 224] = t_feats
    eT = np.ascontiguousarray(e_feats.T)
    in_maps = []
    for ci in range(NCORES):
        m = {
            "x": np.ascontiguousarray(X[ci * BLOC : (ci + 1) * BLOC]),
            "eT": eT,
            "consts": c,
        }
        in_maps.append(m)
    return in_maps


_NC = None


def _get_nc():
    global _NC
    if _NC is None:
        _NC = build_nc()
    return _NC


def kernel(X, t_feats, e_feats):
    in_maps = make_in_maps(X, t_feats, e_feats)
    nc = _get_nc()
    res = run_bass_kernel_spmd(nc, in_maps, list(range(NCORES)))
    out = np.concatenate([res.results[c]["y"] for c in range(NCORES)], axis=0)
    return np.ascontiguousarray(out, dtype=np.float32)
